# revision 23
# baseline (speedup 1.0000x reference)
"""Trainium2 Bass kernel for a 2-bit-quantized (DoReFa) ResNet BasicBlock.

Full (unsharded) numpy inputs -> full numpy output. Internally:
  - batch (64) is sharded 8 images/core across 8 NeuronCores (pure data
    parallel, weights/BN replicated),
  - the 2-bit quantized conv is computed as exact integer arithmetic:
    acts*3 in {0..3}, weights*3 in {-3,-1,1,3} are exact in fp8/bf16 and
    the PE accumulates in fp32, so conv results are bit-exact integers,
  - each 3x3 conv = 9 shifted matmuls accumulated in PSUM over
    zero-padded 30x30 activation planes; with fp8 DoubleRow one matmul
    contracts all 256 input channels (2 interleaved 128-chunks),
  - BN + ReLU + re-quantization folds into 3 per-channel threshold
    compares + 2 adds (masks sum to the quantized integer code),
  - weight quantization (tanh / global max / round) + BN folding is done
    on host: it is O(weights) = 0.6 MB, vs 118 GFLOP of conv on device.
"""

import os
import sys
import numpy as np


def _install_ntff_hook_shim():
    """Provide antenv.axon_hooks if the image lacks it, so
    run_bass_kernel_spmd(trace=True) can capture NTFF profiles through
    libaxon_pjrt.so. No-op if the real module exists or the .so is absent."""
    try:
        import antenv.axon_hooks  # noqa: F401
        return
    except ImportError:
        pass
    import contextlib
    import ctypes
    import types

    so_path = "/opt/axon/libaxon_pjrt.so"
    _hook = None
    if os.path.exists(so_path):
        try:
            lib = ctypes.CDLL(so_path)
        except OSError:
            lib = None
        if lib is not None and hasattr(lib, "axon_start_nrt_profile"):
            lib.axon_start_nrt_profile.argtypes = [
                ctypes.POINTER(ctypes.c_int64), ctypes.c_size_t]
            lib.axon_start_nrt_profile.restype = ctypes.c_int64
            lib.axon_stop_nrt_profile.argtypes = [ctypes.c_char_p]
            lib.axon_stop_nrt_profile.restype = ctypes.c_int64

            @contextlib.contextmanager
            def _hook(output_dir, device_ids):
                import jax
                jax.devices()
                if device_ids:
                    ids = (ctypes.c_int64 * len(device_ids))(*device_ids)
                    rc = lib.axon_start_nrt_profile(ids, len(device_ids))
                else:
                    rc = lib.axon_start_nrt_profile(None, 0)
                if rc != 0:
                    raise RuntimeError(f"axon_start_nrt_profile rc={rc}")
                try:
                    yield
                finally:
                    n = lib.axon_stop_nrt_profile(str(output_dir).encode())
                    print(f"profile: {n} file(s) written to {output_dir}",
                          file=sys.stderr)

    mod = types.ModuleType("antenv.axon_hooks")
    mod.get_axon_ntff_profile_hook = lambda: _hook
    mod.set_axon_ntff_profile_hook = lambda h: None
    sys.modules["antenv.axon_hooks"] = mod


NCORES = 8
NPER = 8          # images per core
C = 256
NCH = 2           # channel chunks of 128
H = W = 28
PIX = H * W
PH = H + 2        # padded plane 30x30
PW = 30           # plane row stride
QSTR = 960        # allocated plane stride (16B-aligned, >= PH*PW)
HALF = 14         # rows per psum tile
RUN = (HALF - 1) * PW + W   # 418-element flat moving-run per matmul
PSF = HALF * PW   # 420 psum columns (cols 28..29 of each row are garbage)
BN_EPS = 1e-5
USE_FP8 = bool(int(os.environ.get("KERNEL_FP8", "1")))


def _quant_weight3(w):
    """Replicate reference _quant_weight in f32, scaled by 3 -> {-3,-1,1,3}."""
    w = np.asarray(w, np.float32)
    t = np.tanh(w)
    m = np.max(np.abs(t))
    t2 = t / (np.float32(2.0) * m) + np.float32(0.5)
    k = np.round(t2 * np.float32(3.0))          # round-half-even == jnp.round
    return (2.0 * k - 3.0).astype(np.float32)


def _fold_bn(g, b, m, v):
    inv = np.asarray(g, np.float64) / np.sqrt(np.asarray(v, np.float64) + BN_EPS)
    beta = np.asarray(b, np.float64) - np.asarray(m, np.float64) * inv
    return inv, beta


def _w_tiles(qw3, dt):
    # [O, I, 3, 3] -> [p=128, ci=2, k=9, O=256] so lhsT slices are [128, 128]
    # (bf16) or [128, 2, 128] interleaved chunks (fp8 DoubleRow).
    return np.ascontiguousarray(
        np.transpose(qw3.reshape(C, NCH, 128, 9), (2, 1, 3, 0))
    ).astype(dt)


def _host_arrays(w1, g1, b1, m1, v1, w2, g2, b2, m2, v2):
    from concourse import mybir
    qw3_1 = _quant_weight3(w1)
    qw3_2 = _quant_weight3(w2)
    inv1, beta1 = _fold_bn(g1, b1, m1, v1)
    inv2, beta2 = _fold_bn(g2, b2, m2, v2)

    act_np = mybir.dt.np(mybir.dt.float8e4 if USE_FP8 else mybir.dt.bfloat16)
    w1t = _w_tiles(qw3_1, act_np)
    w2t = _w_tiles(qw3_2, act_np)

    # conv1 psum P1 (exact int) -> y = P1*inv1/9 + beta1; quant thresholds
    # tau_k=(2k-1)/6 become P1-space thresholds (inv1>0 given g1=1, v1>0).
    assert np.all(inv1 > 0), "bn1 scale must be positive for threshold fold"
    taus = np.array([1.0, 3.0, 5.0]) / 6.0
    t1 = ((taus[None, :] - beta1[:, None]) * 9.0 / inv1[:, None])  # [C, 3]
    t1 = np.ascontiguousarray(
        t1.reshape(NCH, 128, 3).transpose(1, 0, 2)).astype(np.float32)
    # conv2 runs on sign-sums S in {-3..3} with -3 padding: qa' = (S+3)/2
    # everywhere (padding included), so P2 = 0.5*P2s + 1.5*K2f with the
    # per-channel full-tap weight sum K2f -- no border maps needed.
    k2f = qw3_2.reshape(C, -1).sum(axis=1).astype(np.float64)
    s2 = np.ascontiguousarray(
        (inv2 / 18.0).reshape(NCH, 128).T).astype(np.float32)
    bb2 = np.ascontiguousarray(
        (beta2 + 1.5 * k2f * inv2 / 9.0).reshape(NCH, 128).T).astype(np.float32)
    z0 = np.zeros((128, NCH, QSTR), act_np)
    zm3 = np.full((128, NCH, QSTR), -3.0, act_np)
    return {"w1t": w1t, "w2t": w2t, "t1": (-t1).astype(np.float32),
            "s2": s2, "bb2": bb2, "z0": z0, "zm3": zm3}


def _build_program(nper=NPER, stage=3, fp8=USE_FP8):
    from concourse import bacc, tile, mybir
    dt = mybir.dt
    dt_act = dt.float8e4 if fp8 else dt.bfloat16

    nc = bacc.Bacc("TRN2", target_bir_lowering=False, debug=False,
                   num_devices=NCORES)
    NP_ = nper

    x_d = nc.dram_tensor("x", [NP_, C, H, W], dt.float32, kind="ExternalInput")
    w1_d = nc.dram_tensor("w1t", [128, NCH, 9, C], dt_act, kind="ExternalInput")
    w2_d = nc.dram_tensor("w2t", [128, NCH, 9, C], dt_act, kind="ExternalInput")
    t1_d = nc.dram_tensor("t1", [128, NCH, 3], dt.float32, kind="ExternalInput")
    s2_d = nc.dram_tensor("s2", [128, NCH], dt.float32, kind="ExternalInput")
    b2_d = nc.dram_tensor("bb2", [128, NCH], dt.float32, kind="ExternalInput")
    z0_d = nc.dram_tensor("z0", [128, NCH, QSTR], dt_act, kind="ExternalInput")
    zm3_d = nc.dram_tensor("zm3", [128, NCH, QSTR], dt_act,
                           kind="ExternalInput")
    y_d = nc.dram_tensor("y", [NP_, C, H, W], dt.float32, kind="ExternalOutput")

    XQT = [1.0 / 6.0, 3.0 / 6.0, 5.0 / 6.0]   # act-quant thresholds for x

    with tile.TileContext(nc) as tc:
        with (
            tc.tile_pool(name="wpool", bufs=1) as wpool,
            tc.tile_pool(name="xpool", bufs=2 * NP_) as xpool,
            tc.tile_pool(name="qpool", bufs=NP_) as qpool,
            tc.tile_pool(name="mpool", bufs=4) as mpool,
            tc.tile_pool(name="upool", bufs=4) as upool,
            tc.tile_pool(name="opool", bufs=4) as opool,
            tc.tile_pool(name="pspool", bufs=8, space="PSUM") as pspool,
        ):
            w1_sb = wpool.tile([128, NCH, 9, C], dt_act, name="w1sb")
            w2_sb = wpool.tile([128, NCH, 9, C], dt_act, name="w2sb")
            t1_sb = wpool.tile([128, NCH, 3], dt.float32, name="t1sb")
            s2_sb = wpool.tile([128, NCH], dt.float32, name="s2sb")
            b2_sb = wpool.tile([128, NCH], dt.float32, name="b2sb")
            # prologue on the early-starting gpsimd queue: image-0 x loads
            # first (they gate the whole pipeline), then conv1 weights
            warm = wpool.tile([128, 512], dt_act, name="warm")
            nc.gpsimd.memset(warm[:], 1.0)

            # zero-padded quantized-activation planes (flat, per image);
            # qa2 pads with -3: it holds sign-sums S in {-3..3}, and S=-3
            # is exactly the quantized-zero padding the conv needs.
            qa1 = [qpool.tile([128, NCH, QSTR], dt_act, name=f"qa1_{n}",
                              tag="qa1") for n in range(NP_)]
            qa2 = [qpool.tile([128, NCH, QSTR], dt_act, name=f"qa2_{n}",
                              tag="qa2") for n in range(NP_)]

            def plane(qa_t, j):
                return qa_t[:, j, :].rearrange("p (r c) -> p r c", c=PW)

            RA, RB = 16, 12      # image-0 row split: conv1 h=0 needs rows<16
            x0a, x0b = [], []
            for j in range(NCH):
                xta = xpool.tile([128, RA, W], dt.float32, name=f"x0a_{j}",
                                 tag="x0a", bufs=2)
                nc.gpsimd.dma_start(xta[:],
                                    x_d[0, j * 128:(j + 1) * 128, 0:RA, :])
                x0a.append(xta)
            for j in range(NCH):
                xtb = xpool.tile([128, RB, W], dt.float32, name=f"x0b_{j}",
                                 tag="x0b", bufs=2)
                nc.gpsimd.dma_start(xtb[:],
                                    x_d[0, j * 128:(j + 1) * 128, RA:H, :])
                x0b.append(xtb)
            nc.gpsimd.dma_start(qa1[0][:], z0_d[:])
            nc.gpsimd.dma_start(w1_sb[:], w1_d[:])
            nc.gpsimd.dma_start(t1_sb[:], t1_d[:])
            nc.gpsimd.dma_start(w2_sb[:], w2_d[:])
            nc.gpsimd.dma_start(s2_sb[:], s2_d[:])
            nc.gpsimd.dma_start(b2_sb[:], b2_d[:])

            # HAM warm-up: stream of tiny matmuls keeps PE continuously busy
            # from boot so the clock gate is already 8/8 (2.4GHz) when the
            # first real matmuls arrive
            wps = pspool.tile([128, 512], dt.float32, name="wps", tag="ps")
            for _ in range(220):
                nc.tensor.matmul(wps[:, 0:64], warm[:, 0:128], warm[:, 0:64],
                                 start=True, stop=True)

            # load x + quantize into qa1 interiors
            x_sb = [[None] * NCH for _ in range(NP_)]

            def xq_half(n, j, xt, r0, nr):
                # quantize a row-slab of image n chunk j into qa1 padding rows
                mm1 = mpool.tile([128, RA, W], dt.bfloat16, name="mm1",
                                 tag="m1")
                mm2 = mpool.tile([128, RA, W], dt.bfloat16, name="mm2",
                                 tag="m2")
                mm3 = mpool.tile([128, RA, W], dt.bfloat16, name="mm3",
                                 tag="m3")
                mms = mpool.tile([128, RA, W], dt.bfloat16, name="mms",
                                 tag="ms")
                nc.vector.tensor_scalar(mm1[:, 0:nr], xt[:], XQT[0], None,
                                        mybir.AluOpType.is_gt)
                nc.vector.tensor_scalar(mm2[:, 0:nr], xt[:], XQT[1], None,
                                        mybir.AluOpType.is_gt)
                nc.vector.tensor_scalar(mm3[:, 0:nr], xt[:], XQT[2], None,
                                        mybir.AluOpType.is_gt)
                nc.vector.tensor_add(mms[:, 0:nr], mm1[:, 0:nr], mm2[:, 0:nr])
                nc.vector.tensor_add(
                    plane(qa1[n], j)[:, 1 + r0:1 + r0 + nr, 1:1 + W],
                    mms[:, 0:nr], mm3[:, 0:nr])

            def xq_image0_a():
                for j in range(NCH):
                    xq_half(0, j, x0a[j], 0, RA)

            def xq_image0_b():
                for j in range(NCH):
                    xq_half(0, j, x0b[j], RA, RB)
                # full image-0 x for the residual path (late, off the
                # critical ramp)
                for j in range(NCH):
                    xt = xpool.tile([128, H, W], dt.float32, name=f"x_0_{j}",
                                    tag="x")
                    nc.sync.dma_start(xt[:],
                                      x_d[0, j * 128:(j + 1) * 128, :, :])
                    x_sb[0][j] = xt

            def xq_image(n):
                # x loads, then the border fill via DMA: no engine time,
                # no SBUF-port contention (image 0 handled in the prologue)
                if True:
                    for j in range(NCH):
                        xt = xpool.tile([128, H, W], dt.float32,
                                        name=f"x_{n}_{j}", tag="x")
                        nc.sync.dma_start(xt[:],
                                          x_d[n, j * 128:(j + 1) * 128, :, :])
                        x_sb[n][j] = xt
                    nc.sync.dma_start(qa1[n][:], z0_d[:])
                for j in range(NCH):
                    xt = x_sb[n][j]
                    m1 = mpool.tile([128, H, W], dt.bfloat16, name="m1", tag="m1")
                    m2 = mpool.tile([128, H, W], dt.bfloat16, name="m2", tag="m2")
                    m3 = mpool.tile([128, H, W], dt.bfloat16, name="m3", tag="m3")
                    ms = mpool.tile([128, H, W], dt.bfloat16, name="ms", tag="ms")
                    nc.vector.tensor_scalar(m1[:], xt[:], XQT[0], None,
                                            mybir.AluOpType.is_gt)
                    nc.vector.tensor_scalar(m2[:], xt[:], XQT[1], None,
                                            mybir.AluOpType.is_gt)
                    nc.vector.tensor_scalar(m3[:], xt[:], XQT[2], None,
                                            mybir.AluOpType.is_gt)
                    nc.vector.tensor_add(ms[:], m1[:], m2[:])
                    nc.vector.tensor_add(plane(qa1[n], j)[:, 1:1 + H, 1:1 + W],
                                         ms[:], m3[:])

            def dump_qa(qa):
                # debug stages: copy quantized planes out as f32
                for n in range(NP_):
                    for j in range(NCH):
                        o = opool.tile([128, H, W], dt.float32, name="od",
                                       tag="o")
                        nc.vector.tensor_copy(o[:], plane(qa[n], j)[:, 1:1 + H,
                                                                    1:1 + W])
                        nc.sync.dma_start(
                            y_d[n, j * 128:(j + 1) * 128, :, :], o[:])

            def conv_mms(ps, w_sb, qa_n, h, co):
                if fp8:
                    for k in range(9):
                        dy, dx = divmod(k, 3)
                        off = (h * HALF + dy) * PW + dx
                        nc.tensor.matmul(
                            ps[:, 0:RUN],
                            w_sb[:, 0:NCH, k, co * 128:(co + 1) * 128],
                            qa_n[:, 0:NCH, off:off + RUN],
                            start=(k == 0), stop=(k == 8),
                            perf_mode=mybir.MatmulPerfMode.DoubleRow,
                        )
                else:
                    for ci in range(NCH):
                        for k in range(9):
                            dy, dx = divmod(k, 3)
                            nc.tensor.matmul(
                                ps[:],
                                w_sb[:, ci, k, co * 128:(co + 1) * 128],
                                plane(qa_n, ci)[:, h * HALF + dy:
                                                h * HALF + dy + HALF,
                                                dx:dx + W],
                                start=(ci == 0 and k == 0),
                                stop=(ci == NCH - 1 and k == 8),
                            )

            def psum_tile(name):
                if fp8:
                    ps = pspool.tile([128, PSF], dt.float32, name=name,
                                     tag="ps")
                    psv = ps[:].rearrange("p (r c) -> p r c", c=PW)[:, :, 0:W]
                else:
                    ps = pspool.tile([128, HALF, W], dt.float32, name=name,
                                     tag="ps")
                    psv = ps[:]
                return ps, psv

            # conv1 -> bn1 -> relu -> quant; masks as ACT Signs vs folded
            # per-channel thresholds, summed into sign-sums on DVE
            def conv1_tile(n, h, co):
                        ps, psv = psum_tile("ps1")
                        conv_mms(ps, w1_sb, qa1[n], h, co)
                        m1 = mpool.tile([128, HALF, W], dt.bfloat16, name="e1",
                                        tag="e1")
                        m2 = mpool.tile([128, HALF, W], dt.bfloat16, name="e2",
                                        tag="e2")
                        m3 = mpool.tile([128, HALF, W], dt.bfloat16, name="e3",
                                        tag="e3")
                        ms = mpool.tile([128, HALF, W], dt.bfloat16, name="es",
                                        tag="es")
                        for k, mk in enumerate((m1, m2, m3)):
                            nc.scalar.activation(
                                mk[:], psv, mybir.ActivationFunctionType.Sign,
                                bias=t1_sb[:, co, k:k + 1])
                        nc.vector.tensor_add(ms[:], m1[:], m2[:])
                        nc.vector.tensor_add(
                            plane(qa2[n], co)[:, 1 + h * HALF:
                                              1 + h * HALF + HALF, 1:1 + W],
                            ms[:], m3[:])

            def conv1_image(n):
                nc.sync.dma_start(qa2[n][:], zm3_d[:])
                for h in range(2):
                    for co in range(NCH):
                        conv1_tile(n, h, co)

            # conv2 -> bn2 -> +residual -> relu -> out
            def conv2_image(n):
                for h in range(2):
                    for co in range(NCH):
                        ps, psv = psum_tile("ps2")
                        conv_mms(ps, w2_sb, qa2[n], h, co)
                        u = upool.tile([128, HALF, W], dt.float32, name="u",
                                       tag="u")
                        v = upool.tile([128, HALF, W], dt.float32, name="v",
                                       tag="v")
                        o = opool.tile([128, HALF, W], dt.float32, name="o",
                                       tag="o")
                        nc.scalar.activation(
                            u[:], psv,
                            mybir.ActivationFunctionType.Identity,
                            bias=b2_sb[:, co:co + 1], scale=s2_sb[:, co:co + 1])
                        nc.vector.tensor_add(
                            v[:], u[:],
                            x_sb[n][co][:, h * HALF:(h + 1) * HALF, :])
                        nc.scalar.activation(
                            o[:], v[:], mybir.ActivationFunctionType.Relu)
                        nc.sync.dma_start(
                            y_d[n, co * 128:(co + 1) * 128,
                                h * HALF:(h + 1) * HALF, :],
                            o[:])

            def conv1_half(n, h):
                for co in range(NCH):
                    conv1_tile(n, h, co)

            # software-pipelined emission: decouple engine queues by image
            for n in range(NP_):
                if n == 0:
                    nc.sync.dma_start(qa2[0][:], zm3_d[:])
                    xq_image0_a()
                    if stage >= 2:
                        conv1_half(0, 0)
                    xq_image0_b()
                    if stage >= 2:
                        conv1_half(0, 1)
                else:
                    xq_image(n)
                    if stage >= 2:
                        conv1_image(n)
                if stage >= 3 and n >= 1:
                    conv2_image(n - 1)
            if stage == 1:
                dump_qa(qa1)
            if stage == 2:
                dump_qa(qa2)
            if stage >= 3:
                conv2_image(NP_ - 1)

    nc.compile()
    return nc


_CACHED = None


def _get_program():
    global _CACHED
    if _CACHED is None:
        _CACHED = _build_program()
    return _CACHED


def kernel(x, w1, g1, b1, m1, v1, w2, g2, b2, m2, v2):
    _install_ntff_hook_shim()
    from concourse.bass_utils import run_bass_kernel_spmd

    x = np.asarray(x, np.float32)
    host = _host_arrays(w1, g1, b1, m1, v1, w2, g2, b2, m2, v2)

    xs = x.reshape(NCORES, NPER, C, H, W)
    in_maps = [{"x": np.ascontiguousarray(xs[c]), **host}
               for c in range(NCORES)]

    nc = _get_program()
    res = run_bass_kernel_spmd(
        nc, in_maps, core_ids=list(range(NCORES)),
        trace=bool(int(os.environ.get("KERNEL_TRACE", "0"))),
    )
    kernel.last_results = res
    y = np.concatenate([res.results[c]["y"][None] for c in range(NCORES)], 0)
    return np.ascontiguousarray(y.reshape(64, C, H, W).astype(np.float32))


# revision 24
# speedup vs baseline: 1.0162x; 1.0162x over previous
"""Trainium2 Bass kernel for a 2-bit-quantized (DoReFa) ResNet BasicBlock.

Full (unsharded) numpy inputs -> full numpy output. Internally:
  - batch (64) is sharded 8 images/core across 8 NeuronCores (pure data
    parallel, weights/BN replicated),
  - the 2-bit quantized conv is computed as exact integer arithmetic:
    acts*3 in {0..3}, weights*3 in {-3,-1,1,3} are exact in fp8/bf16 and
    the PE accumulates in fp32, so conv results are bit-exact integers,
  - each 3x3 conv = 9 shifted matmuls accumulated in PSUM over
    zero-padded 30x30 activation planes; with fp8 DoubleRow one matmul
    contracts all 256 input channels (2 interleaved 128-chunks),
  - BN + ReLU + re-quantization folds into 3 per-channel threshold
    compares + 2 adds (masks sum to the quantized integer code),
  - weight quantization (tanh / global max / round) + BN folding is done
    on host: it is O(weights) = 0.6 MB, vs 118 GFLOP of conv on device.
"""

import os
import sys
import numpy as np


def _install_ntff_hook_shim():
    """Provide antenv.axon_hooks if the image lacks it, so
    run_bass_kernel_spmd(trace=True) can capture NTFF profiles through
    libaxon_pjrt.so. No-op if the real module exists or the .so is absent."""
    try:
        import antenv.axon_hooks  # noqa: F401
        return
    except ImportError:
        pass
    import contextlib
    import ctypes
    import types

    so_path = "/opt/axon/libaxon_pjrt.so"
    _hook = None
    if os.path.exists(so_path):
        try:
            lib = ctypes.CDLL(so_path)
        except OSError:
            lib = None
        if lib is not None and hasattr(lib, "axon_start_nrt_profile"):
            lib.axon_start_nrt_profile.argtypes = [
                ctypes.POINTER(ctypes.c_int64), ctypes.c_size_t]
            lib.axon_start_nrt_profile.restype = ctypes.c_int64
            lib.axon_stop_nrt_profile.argtypes = [ctypes.c_char_p]
            lib.axon_stop_nrt_profile.restype = ctypes.c_int64

            @contextlib.contextmanager
            def _hook(output_dir, device_ids):
                import jax
                jax.devices()
                if device_ids:
                    ids = (ctypes.c_int64 * len(device_ids))(*device_ids)
                    rc = lib.axon_start_nrt_profile(ids, len(device_ids))
                else:
                    rc = lib.axon_start_nrt_profile(None, 0)
                if rc != 0:
                    raise RuntimeError(f"axon_start_nrt_profile rc={rc}")
                try:
                    yield
                finally:
                    n = lib.axon_stop_nrt_profile(str(output_dir).encode())
                    print(f"profile: {n} file(s) written to {output_dir}",
                          file=sys.stderr)

    mod = types.ModuleType("antenv.axon_hooks")
    mod.get_axon_ntff_profile_hook = lambda: _hook
    mod.set_axon_ntff_profile_hook = lambda h: None
    sys.modules["antenv.axon_hooks"] = mod


NCORES = 8
NPER = 8          # images per core
C = 256
NCH = 2           # channel chunks of 128
H = W = 28
PIX = H * W
PH = H + 2        # padded plane 30x30
PW = 30           # plane row stride
QSTR = 960        # allocated plane stride (16B-aligned, >= PH*PW)
HALF = 14         # rows per psum tile
RUN = (HALF - 1) * PW + W   # 418-element flat moving-run per matmul
PSF = HALF * PW   # 420 psum columns (cols 28..29 of each row are garbage)
BN_EPS = 1e-5
USE_FP8 = bool(int(os.environ.get("KERNEL_FP8", "1")))


def _quant_weight3(w):
    """Replicate reference _quant_weight in f32, scaled by 3 -> {-3,-1,1,3}."""
    w = np.asarray(w, np.float32)
    t = np.tanh(w)
    m = np.max(np.abs(t))
    t2 = t / (np.float32(2.0) * m) + np.float32(0.5)
    k = np.round(t2 * np.float32(3.0))          # round-half-even == jnp.round
    return (2.0 * k - 3.0).astype(np.float32)


def _fold_bn(g, b, m, v):
    inv = np.asarray(g, np.float64) / np.sqrt(np.asarray(v, np.float64) + BN_EPS)
    beta = np.asarray(b, np.float64) - np.asarray(m, np.float64) * inv
    return inv, beta


def _w_tiles(qw3, dt):
    # [O, I, 3, 3] -> [p=128, ci=2, k=9, O=256] so lhsT slices are [128, 128]
    # (bf16) or [128, 2, 128] interleaved chunks (fp8 DoubleRow).
    return np.ascontiguousarray(
        np.transpose(qw3.reshape(C, NCH, 128, 9), (2, 1, 3, 0))
    ).astype(dt)


def _host_arrays(w1, g1, b1, m1, v1, w2, g2, b2, m2, v2):
    from concourse import mybir
    qw3_1 = _quant_weight3(w1)
    qw3_2 = _quant_weight3(w2)
    inv1, beta1 = _fold_bn(g1, b1, m1, v1)
    inv2, beta2 = _fold_bn(g2, b2, m2, v2)

    act_np = mybir.dt.np(mybir.dt.float8e4 if USE_FP8 else mybir.dt.bfloat16)
    w1t = _w_tiles(qw3_1, act_np)
    w2t = _w_tiles(qw3_2, act_np)

    # conv1 psum P1 (exact int) -> y = P1*inv1/9 + beta1; quant thresholds
    # tau_k=(2k-1)/6 become P1-space thresholds (inv1>0 given g1=1, v1>0).
    assert np.all(inv1 > 0), "bn1 scale must be positive for threshold fold"
    taus = np.array([1.0, 3.0, 5.0]) / 6.0
    t1 = ((taus[None, :] - beta1[:, None]) * 9.0 / inv1[:, None])  # [C, 3]
    t1 = np.ascontiguousarray(
        t1.reshape(NCH, 128, 3).transpose(1, 0, 2)).astype(np.float32)
    # conv2 runs on sign-sums S in {-3..3} with -3 padding: qa' = (S+3)/2
    # everywhere (padding included), so P2 = 0.5*P2s + 1.5*K2f with the
    # per-channel full-tap weight sum K2f -- no border maps needed.
    k2f = qw3_2.reshape(C, -1).sum(axis=1).astype(np.float64)
    s2 = np.ascontiguousarray(
        (inv2 / 18.0).reshape(NCH, 128).T).astype(np.float32)
    bb2 = np.ascontiguousarray(
        (beta2 + 1.5 * k2f * inv2 / 9.0).reshape(NCH, 128).T).astype(np.float32)
    z0 = np.zeros((128, NCH, QSTR), act_np)
    zm3 = np.full((128, NCH, QSTR), -3.0, act_np)
    return {"w1t": w1t, "w2t": w2t, "t1": (-t1).astype(np.float32),
            "s2": s2, "bb2": bb2, "z0": z0, "zm3": zm3}


def _build_program(nper=NPER, stage=3, fp8=USE_FP8):
    from concourse import bacc, tile, mybir
    dt = mybir.dt
    dt_act = dt.float8e4 if fp8 else dt.bfloat16

    nc = bacc.Bacc("TRN2", target_bir_lowering=False, debug=False,
                   num_devices=NCORES)
    NP_ = nper

    x_d = nc.dram_tensor("x", [NP_, C, H, W], dt.float32, kind="ExternalInput")
    w1_d = nc.dram_tensor("w1t", [128, NCH, 9, C], dt_act, kind="ExternalInput")
    w2_d = nc.dram_tensor("w2t", [128, NCH, 9, C], dt_act, kind="ExternalInput")
    t1_d = nc.dram_tensor("t1", [128, NCH, 3], dt.float32, kind="ExternalInput")
    s2_d = nc.dram_tensor("s2", [128, NCH], dt.float32, kind="ExternalInput")
    b2_d = nc.dram_tensor("bb2", [128, NCH], dt.float32, kind="ExternalInput")
    z0_d = nc.dram_tensor("z0", [128, NCH, QSTR], dt_act, kind="ExternalInput")
    zm3_d = nc.dram_tensor("zm3", [128, NCH, QSTR], dt_act,
                           kind="ExternalInput")
    y_d = nc.dram_tensor("y", [NP_, C, H, W], dt.float32, kind="ExternalOutput")

    XQT = [1.0 / 6.0, 3.0 / 6.0, 5.0 / 6.0]   # act-quant thresholds for x

    with tile.TileContext(nc) as tc:
        with (
            tc.tile_pool(name="wpool", bufs=1) as wpool,
            tc.tile_pool(name="xpool", bufs=2 * NP_) as xpool,
            tc.tile_pool(name="qpool", bufs=NP_) as qpool,
            tc.tile_pool(name="mpool", bufs=4) as mpool,
            tc.tile_pool(name="upool", bufs=4) as upool,
            tc.tile_pool(name="opool", bufs=4) as opool,
            tc.tile_pool(name="pspool", bufs=8, space="PSUM") as pspool,
        ):
            w1_sb = wpool.tile([128, NCH, 9, C], dt_act, name="w1sb")
            w2_sb = wpool.tile([128, NCH, 9, C], dt_act, name="w2sb")
            t1_sb = wpool.tile([128, NCH, 3], dt.float32, name="t1sb")
            s2_sb = wpool.tile([128, NCH], dt.float32, name="s2sb")
            b2_sb = wpool.tile([128, NCH], dt.float32, name="b2sb")
            # prologue on the early-starting gpsimd queue: image-0 x loads
            # first (they gate the whole pipeline), then conv1 weights
            warm = wpool.tile([128, 512], dt_act, name="warm")
            nc.gpsimd.memset(warm[:], 1.0)

            # zero-padded quantized-activation planes (flat, per image);
            # qa2 pads with -3: it holds sign-sums S in {-3..3}, and S=-3
            # is exactly the quantized-zero padding the conv needs.
            qa1 = [qpool.tile([128, NCH, QSTR], dt_act, name=f"qa1_{n}",
                              tag="qa1") for n in range(NP_)]
            qa2 = [qpool.tile([128, NCH, QSTR], dt_act, name=f"qa2_{n}",
                              tag="qa2") for n in range(NP_)]

            def plane(qa_t, j):
                return qa_t[:, j, :].rearrange("p (r c) -> p r c", c=PW)

            # spread the ramp-critical transfers over BOTH dma paths so
            # they flow concurrently once the dma subsystem comes up
            RA, RB = 16, 12      # image-0 row split: conv1 h=0 needs rows<16
            x0a, x0b = [], []
            for j in range(NCH):
                xta = xpool.tile([128, RA, W], dt.float32, name=f"x0a_{j}",
                                 tag="x0a", bufs=2)
                nc.gpsimd.dma_start(xta[:],
                                    x_d[0, j * 128:(j + 1) * 128, 0:RA, :])
                x0a.append(xta)
            nc.sync.dma_start(qa1[0][:], z0_d[:])
            nc.sync.dma_start(w1_sb[:], w1_d[:])
            for j in range(NCH):
                xtb = xpool.tile([128, RB, W], dt.float32, name=f"x0b_{j}",
                                 tag="x0b", bufs=2)
                nc.gpsimd.dma_start(xtb[:],
                                    x_d[0, j * 128:(j + 1) * 128, RA:H, :])
                x0b.append(xtb)
            nc.sync.dma_start(t1_sb[:], t1_d[:])
            nc.gpsimd.dma_start(w2_sb[:], w2_d[:])
            nc.gpsimd.dma_start(s2_sb[:], s2_d[:])
            nc.gpsimd.dma_start(b2_sb[:], b2_d[:])

            # HAM warm-up: stream of tiny matmuls keeps PE continuously busy
            # from boot so the clock gate is already 8/8 (2.4GHz) when the
            # first real matmuls arrive
            wps = pspool.tile([128, 512], dt.float32, name="wps", tag="ps")
            for _ in range(130):
                nc.tensor.matmul(wps[:, 0:64], warm[:, 0:128], warm[:, 0:64],
                                 start=True, stop=True)

            # load x + quantize into qa1 interiors
            x_sb = [[None] * NCH for _ in range(NP_)]

            def xq_half(n, j, xt, r0, nr):
                # quantize a row-slab of image n chunk j into qa1 padding rows
                mm1 = mpool.tile([128, RA, W], dt.bfloat16, name="mm1",
                                 tag="m1")
                mm2 = mpool.tile([128, RA, W], dt.bfloat16, name="mm2",
                                 tag="m2")
                mm3 = mpool.tile([128, RA, W], dt.bfloat16, name="mm3",
                                 tag="m3")
                mms = mpool.tile([128, RA, W], dt.bfloat16, name="mms",
                                 tag="ms")
                nc.vector.tensor_scalar(mm1[:, 0:nr], xt[:], XQT[0], None,
                                        mybir.AluOpType.is_gt)
                nc.vector.tensor_scalar(mm2[:, 0:nr], xt[:], XQT[1], None,
                                        mybir.AluOpType.is_gt)
                nc.vector.tensor_scalar(mm3[:, 0:nr], xt[:], XQT[2], None,
                                        mybir.AluOpType.is_gt)
                nc.vector.tensor_add(mms[:, 0:nr], mm1[:, 0:nr], mm2[:, 0:nr])
                nc.vector.tensor_add(
                    plane(qa1[n], j)[:, 1 + r0:1 + r0 + nr, 1:1 + W],
                    mms[:, 0:nr], mm3[:, 0:nr])

            def xq_image0_a():
                for j in range(NCH):
                    xq_half(0, j, x0a[j], 0, RA)

            def xq_image0_b():
                for j in range(NCH):
                    xq_half(0, j, x0b[j], RA, RB)
                # full image-0 x for the residual path (late, off the
                # critical ramp)
                for j in range(NCH):
                    xt = xpool.tile([128, H, W], dt.float32, name=f"x_0_{j}",
                                    tag="x")
                    nc.sync.dma_start(xt[:],
                                      x_d[0, j * 128:(j + 1) * 128, :, :])
                    x_sb[0][j] = xt

            def xq_image(n):
                # x loads, then the border fill via DMA: no engine time,
                # no SBUF-port contention (image 0 handled in the prologue)
                if True:
                    for j in range(NCH):
                        xt = xpool.tile([128, H, W], dt.float32,
                                        name=f"x_{n}_{j}", tag="x")
                        nc.sync.dma_start(xt[:],
                                          x_d[n, j * 128:(j + 1) * 128, :, :])
                        x_sb[n][j] = xt
                    nc.sync.dma_start(qa1[n][:], z0_d[:])
                for j in range(NCH):
                    xt = x_sb[n][j]
                    m1 = mpool.tile([128, H, W], dt.bfloat16, name="m1", tag="m1")
                    m2 = mpool.tile([128, H, W], dt.bfloat16, name="m2", tag="m2")
                    m3 = mpool.tile([128, H, W], dt.bfloat16, name="m3", tag="m3")
                    ms = mpool.tile([128, H, W], dt.bfloat16, name="ms", tag="ms")
                    nc.vector.tensor_scalar(m1[:], xt[:], XQT[0], None,
                                            mybir.AluOpType.is_gt)
                    nc.vector.tensor_scalar(m2[:], xt[:], XQT[1], None,
                                            mybir.AluOpType.is_gt)
                    nc.vector.tensor_scalar(m3[:], xt[:], XQT[2], None,
                                            mybir.AluOpType.is_gt)
                    nc.vector.tensor_add(ms[:], m1[:], m2[:])
                    nc.vector.tensor_add(plane(qa1[n], j)[:, 1:1 + H, 1:1 + W],
                                         ms[:], m3[:])

            def dump_qa(qa):
                # debug stages: copy quantized planes out as f32
                for n in range(NP_):
                    for j in range(NCH):
                        o = opool.tile([128, H, W], dt.float32, name="od",
                                       tag="o")
                        nc.vector.tensor_copy(o[:], plane(qa[n], j)[:, 1:1 + H,
                                                                    1:1 + W])
                        nc.sync.dma_start(
                            y_d[n, j * 128:(j + 1) * 128, :, :], o[:])

            def conv_mms(ps, w_sb, qa_n, h, co):
                if fp8:
                    for k in range(9):
                        dy, dx = divmod(k, 3)
                        off = (h * HALF + dy) * PW + dx
                        nc.tensor.matmul(
                            ps[:, 0:RUN],
                            w_sb[:, 0:NCH, k, co * 128:(co + 1) * 128],
                            qa_n[:, 0:NCH, off:off + RUN],
                            start=(k == 0), stop=(k == 8),
                            perf_mode=mybir.MatmulPerfMode.DoubleRow,
                        )
                else:
                    for ci in range(NCH):
                        for k in range(9):
                            dy, dx = divmod(k, 3)
                            nc.tensor.matmul(
                                ps[:],
                                w_sb[:, ci, k, co * 128:(co + 1) * 128],
                                plane(qa_n, ci)[:, h * HALF + dy:
                                                h * HALF + dy + HALF,
                                                dx:dx + W],
                                start=(ci == 0 and k == 0),
                                stop=(ci == NCH - 1 and k == 8),
                            )

            def psum_tile(name):
                if fp8:
                    ps = pspool.tile([128, PSF], dt.float32, name=name,
                                     tag="ps")
                    psv = ps[:].rearrange("p (r c) -> p r c", c=PW)[:, :, 0:W]
                else:
                    ps = pspool.tile([128, HALF, W], dt.float32, name=name,
                                     tag="ps")
                    psv = ps[:]
                return ps, psv

            # conv1 -> bn1 -> relu -> quant; masks as ACT Signs vs folded
            # per-channel thresholds, summed into sign-sums on DVE
            def conv1_tile(n, h, co):
                        ps, psv = psum_tile("ps1")
                        conv_mms(ps, w1_sb, qa1[n], h, co)
                        m1 = mpool.tile([128, HALF, W], dt.bfloat16, name="e1",
                                        tag="e1")
                        m2 = mpool.tile([128, HALF, W], dt.bfloat16, name="e2",
                                        tag="e2")
                        m3 = mpool.tile([128, HALF, W], dt.bfloat16, name="e3",
                                        tag="e3")
                        ms = mpool.tile([128, HALF, W], dt.bfloat16, name="es",
                                        tag="es")
                        for k, mk in enumerate((m1, m2, m3)):
                            nc.scalar.activation(
                                mk[:], psv, mybir.ActivationFunctionType.Sign,
                                bias=t1_sb[:, co, k:k + 1])
                        nc.vector.tensor_add(ms[:], m1[:], m2[:])
                        nc.vector.tensor_add(
                            plane(qa2[n], co)[:, 1 + h * HALF:
                                              1 + h * HALF + HALF, 1:1 + W],
                            ms[:], m3[:])

            def conv1_image(n):
                nc.sync.dma_start(qa2[n][:], zm3_d[:])
                for h in range(2):
                    for co in range(NCH):
                        conv1_tile(n, h, co)

            # conv2 -> bn2 -> +residual -> relu -> out
            def conv2_image(n):
                for h in range(2):
                    for co in range(NCH):
                        ps, psv = psum_tile("ps2")
                        conv_mms(ps, w2_sb, qa2[n], h, co)
                        u = upool.tile([128, HALF, W], dt.float32, name="u",
                                       tag="u")
                        v = upool.tile([128, HALF, W], dt.float32, name="v",
                                       tag="v")
                        o = opool.tile([128, HALF, W], dt.float32, name="o",
                                       tag="o")
                        nc.scalar.activation(
                            u[:], psv,
                            mybir.ActivationFunctionType.Identity,
                            bias=b2_sb[:, co:co + 1], scale=s2_sb[:, co:co + 1])
                        nc.vector.tensor_add(
                            v[:], u[:],
                            x_sb[n][co][:, h * HALF:(h + 1) * HALF, :])
                        nc.scalar.activation(
                            o[:], v[:], mybir.ActivationFunctionType.Relu)
                        nc.sync.dma_start(
                            y_d[n, co * 128:(co + 1) * 128,
                                h * HALF:(h + 1) * HALF, :],
                            o[:])

            def conv1_half(n, h):
                for co in range(NCH):
                    conv1_tile(n, h, co)

            # software-pipelined emission: decouple engine queues by image
            for n in range(NP_):
                if n == 0:
                    nc.sync.dma_start(qa2[0][:], zm3_d[:])
                    xq_image0_a()
                    if stage >= 2:
                        conv1_half(0, 0)
                    xq_image0_b()
                    if stage >= 2:
                        conv1_half(0, 1)
                else:
                    xq_image(n)
                    if stage >= 2:
                        conv1_image(n)
                if stage >= 3 and n >= 1:
                    conv2_image(n - 1)
            if stage == 1:
                dump_qa(qa1)
            if stage == 2:
                dump_qa(qa2)
            if stage >= 3:
                conv2_image(NP_ - 1)

    nc.compile()
    return nc


_CACHED = None


def _get_program():
    global _CACHED
    if _CACHED is None:
        _CACHED = _build_program()
    return _CACHED


def kernel(x, w1, g1, b1, m1, v1, w2, g2, b2, m2, v2):
    _install_ntff_hook_shim()
    from concourse.bass_utils import run_bass_kernel_spmd

    x = np.asarray(x, np.float32)
    host = _host_arrays(w1, g1, b1, m1, v1, w2, g2, b2, m2, v2)

    xs = x.reshape(NCORES, NPER, C, H, W)
    in_maps = [{"x": np.ascontiguousarray(xs[c]), **host}
               for c in range(NCORES)]

    nc = _get_program()
    res = run_bass_kernel_spmd(
        nc, in_maps, core_ids=list(range(NCORES)),
        trace=bool(int(os.environ.get("KERNEL_TRACE", "0"))),
    )
    kernel.last_results = res
    y = np.concatenate([res.results[c]["y"][None] for c in range(NCORES)], 0)
    return np.ascontiguousarray(y.reshape(64, C, H, W).astype(np.float32))


# revision 26
# speedup vs baseline: 1.0549x; 1.0381x over previous
"""Trainium2 Bass kernel for a 2-bit-quantized (DoReFa) ResNet BasicBlock.

Full (unsharded) numpy inputs -> full numpy output. Internally:
  - batch (64) is sharded 8 images/core across 8 NeuronCores (pure data
    parallel, weights/BN replicated),
  - the 2-bit quantized conv is computed as exact integer arithmetic:
    acts*3 in {0..3}, weights*3 in {-3,-1,1,3} are exact in fp8/bf16 and
    the PE accumulates in fp32, so conv results are bit-exact integers,
  - each 3x3 conv = 9 shifted matmuls accumulated in PSUM over
    zero-padded 30x30 activation planes; with fp8 DoubleRow one matmul
    contracts all 256 input channels (2 interleaved 128-chunks),
  - BN + ReLU + re-quantization folds into 3 per-channel threshold
    compares + 2 adds (masks sum to the quantized integer code),
  - weight quantization (tanh / global max / round) + BN folding is done
    on host: it is O(weights) = 0.6 MB, vs 118 GFLOP of conv on device.
"""

import os
import sys
import numpy as np


def _install_ntff_hook_shim():
    """Provide antenv.axon_hooks if the image lacks it, so
    run_bass_kernel_spmd(trace=True) can capture NTFF profiles through
    libaxon_pjrt.so. No-op if the real module exists or the .so is absent."""
    try:
        import antenv.axon_hooks  # noqa: F401
        return
    except ImportError:
        pass
    import contextlib
    import ctypes
    import types

    so_path = "/opt/axon/libaxon_pjrt.so"
    _hook = None
    if os.path.exists(so_path):
        try:
            lib = ctypes.CDLL(so_path)
        except OSError:
            lib = None
        if lib is not None and hasattr(lib, "axon_start_nrt_profile"):
            lib.axon_start_nrt_profile.argtypes = [
                ctypes.POINTER(ctypes.c_int64), ctypes.c_size_t]
            lib.axon_start_nrt_profile.restype = ctypes.c_int64
            lib.axon_stop_nrt_profile.argtypes = [ctypes.c_char_p]
            lib.axon_stop_nrt_profile.restype = ctypes.c_int64

            @contextlib.contextmanager
            def _hook(output_dir, device_ids):
                import jax
                jax.devices()
                if device_ids:
                    ids = (ctypes.c_int64 * len(device_ids))(*device_ids)
                    rc = lib.axon_start_nrt_profile(ids, len(device_ids))
                else:
                    rc = lib.axon_start_nrt_profile(None, 0)
                if rc != 0:
                    raise RuntimeError(f"axon_start_nrt_profile rc={rc}")
                try:
                    yield
                finally:
                    n = lib.axon_stop_nrt_profile(str(output_dir).encode())
                    print(f"profile: {n} file(s) written to {output_dir}",
                          file=sys.stderr)

    mod = types.ModuleType("antenv.axon_hooks")
    mod.get_axon_ntff_profile_hook = lambda: _hook
    mod.set_axon_ntff_profile_hook = lambda h: None
    sys.modules["antenv.axon_hooks"] = mod


NCORES = 8
NPER = 8          # images per core
C = 256
NCH = 2           # channel chunks of 128
H = W = 28
PIX = H * W
PH = H + 2        # padded plane 30x30
PW = 30           # plane row stride
QSTR = 960        # allocated plane stride (16B-aligned, >= PH*PW)
HALF = 14         # rows per psum tile
RUN = (HALF - 1) * PW + W   # 418-element flat moving-run per matmul
PSF = HALF * PW   # 420 psum columns (cols 28..29 of each row are garbage)
BN_EPS = 1e-5
USE_FP8 = bool(int(os.environ.get("KERNEL_FP8", "1")))


def _quant_weight3(w):
    """Replicate reference _quant_weight in f32, scaled by 3 -> {-3,-1,1,3}."""
    w = np.asarray(w, np.float32)
    t = np.tanh(w)
    m = np.max(np.abs(t))
    t2 = t / (np.float32(2.0) * m) + np.float32(0.5)
    k = np.round(t2 * np.float32(3.0))          # round-half-even == jnp.round
    return (2.0 * k - 3.0).astype(np.float32)


def _fold_bn(g, b, m, v):
    inv = np.asarray(g, np.float64) / np.sqrt(np.asarray(v, np.float64) + BN_EPS)
    beta = np.asarray(b, np.float64) - np.asarray(m, np.float64) * inv
    return inv, beta


def _w_tiles(qw3, dt):
    # [O, I, 3, 3] -> [p=128, ci=2, k=9, O=256] so lhsT slices are [128, 128]
    # (bf16) or [128, 2, 128] interleaved chunks (fp8 DoubleRow).
    return np.ascontiguousarray(
        np.transpose(qw3.reshape(C, NCH, 128, 9), (2, 1, 3, 0))
    ).astype(dt)


def _host_arrays(w1, g1, b1, m1, v1, w2, g2, b2, m2, v2):
    from concourse import mybir
    qw3_1 = _quant_weight3(w1)
    qw3_2 = _quant_weight3(w2)
    inv1, beta1 = _fold_bn(g1, b1, m1, v1)
    inv2, beta2 = _fold_bn(g2, b2, m2, v2)

    act_np = mybir.dt.np(mybir.dt.float8e4 if USE_FP8 else mybir.dt.bfloat16)
    w1t = _w_tiles(qw3_1, act_np)
    w2t = _w_tiles(qw3_2, act_np)

    # conv1 psum P1 (exact int) -> y = P1*inv1/9 + beta1; quant thresholds
    # tau_k=(2k-1)/6 become P1-space thresholds (inv1>0 given g1=1, v1>0).
    assert np.all(inv1 > 0), "bn1 scale must be positive for threshold fold"
    taus = np.array([1.0, 3.0, 5.0]) / 6.0
    t1 = ((taus[None, :] - beta1[:, None]) * 9.0 / inv1[:, None])  # [C, 3]
    t1 = np.ascontiguousarray(
        t1.reshape(NCH, 128, 3).transpose(1, 0, 2)).astype(np.float32)
    # conv2 runs on sign-sums S in {-3..3} with -3 padding: qa' = (S+3)/2
    # everywhere (padding included), so P2 = 0.5*P2s + 1.5*K2f with the
    # per-channel full-tap weight sum K2f -- no border maps needed.
    k2f = qw3_2.reshape(C, -1).sum(axis=1).astype(np.float64)
    s2 = np.ascontiguousarray(
        (inv2 / 18.0).reshape(NCH, 128).T).astype(np.float32)
    bb2 = np.ascontiguousarray(
        (beta2 + 1.5 * k2f * inv2 / 9.0).reshape(NCH, 128).T).astype(np.float32)
    z0 = np.zeros((128, NCH, QSTR), act_np)
    zm3 = np.full((128, NCH, QSTR), -3.0, act_np)
    return {"w1t": w1t, "w2t": w2t, "t1": (-t1).astype(np.float32),
            "s2": s2, "bb2": bb2, "z0": z0, "zm3": zm3}


def _build_program(nper=NPER, stage=3, fp8=USE_FP8):
    from concourse import bacc, tile, mybir
    dt = mybir.dt
    dt_act = dt.float8e4 if fp8 else dt.bfloat16

    nc = bacc.Bacc("TRN2", target_bir_lowering=False, debug=False,
                   num_devices=NCORES)
    NP_ = nper

    x_d = nc.dram_tensor("x", [NP_, C, H, W], dt.float32, kind="ExternalInput")
    w1_d = nc.dram_tensor("w1t", [128, NCH, 9, C], dt_act, kind="ExternalInput")
    w2_d = nc.dram_tensor("w2t", [128, NCH, 9, C], dt_act, kind="ExternalInput")
    t1_d = nc.dram_tensor("t1", [128, NCH, 3], dt.float32, kind="ExternalInput")
    s2_d = nc.dram_tensor("s2", [128, NCH], dt.float32, kind="ExternalInput")
    b2_d = nc.dram_tensor("bb2", [128, NCH], dt.float32, kind="ExternalInput")
    z0_d = nc.dram_tensor("z0", [128, NCH, QSTR], dt_act, kind="ExternalInput")
    zm3_d = nc.dram_tensor("zm3", [128, NCH, QSTR], dt_act,
                           kind="ExternalInput")
    y_d = nc.dram_tensor("y", [NP_, C, H, W], dt.float32, kind="ExternalOutput")

    XQT = [1.0 / 6.0, 3.0 / 6.0, 5.0 / 6.0]   # act-quant thresholds for x

    with tile.TileContext(nc) as tc:
        with (
            tc.tile_pool(name="wpool", bufs=1) as wpool,
            tc.tile_pool(name="xpool", bufs=2 * NP_) as xpool,
            tc.tile_pool(name="qpool", bufs=NP_) as qpool,
            tc.tile_pool(name="mpool", bufs=4) as mpool,
            tc.tile_pool(name="upool", bufs=4) as upool,
            tc.tile_pool(name="opool", bufs=4) as opool,
            tc.tile_pool(name="pspool", bufs=8, space="PSUM") as pspool,
        ):
            w1_sb = wpool.tile([128, NCH, 9, C], dt_act, name="w1sb")
            w2_sb = wpool.tile([128, NCH, 9, C], dt_act, name="w2sb")
            t1_sb = wpool.tile([128, NCH, 3], dt.float32, name="t1sb")
            s2_sb = wpool.tile([128, NCH], dt.float32, name="s2sb")
            b2_sb = wpool.tile([128, NCH], dt.float32, name="b2sb")
            # zero-padded quantized-activation planes (flat, per image);
            # qa2 pads with -3: it holds sign-sums S in {-3..3}, and S=-3
            # is exactly the quantized-zero padding the conv needs.
            qa1 = [qpool.tile([128, NCH, QSTR], dt_act, name=f"qa1_{n}",
                              tag="qa1") for n in range(NP_)]
            qa2 = [qpool.tile([128, NCH, QSTR], dt_act, name=f"qa2_{n}",
                              tag="qa2") for n in range(NP_)]

            def plane(qa_t, j):
                return qa_t[:, j, :].rearrange("p (r c) -> p r c", c=PW)

            # params on the gpsimd DMA queue so x loads own the sync queue
            nc.gpsimd.dma_start(w1_sb[:], w1_d[:])
            nc.gpsimd.dma_start(w2_sb[:], w2_d[:])
            nc.gpsimd.dma_start(t1_sb[:], t1_d[:])
            nc.gpsimd.dma_start(s2_sb[:], s2_d[:])
            nc.gpsimd.dma_start(b2_sb[:], b2_d[:])

            # load x + quantize into qa1 interiors
            x_sb = [[None] * NCH for _ in range(NP_)]

            def xq_image(n):
                # x loads, then the border fill via DMA: no engine time,
                # no SBUF-port contention
                for j in range(NCH):
                    xt = xpool.tile([128, H, W], dt.float32,
                                    name=f"x_{n}_{j}", tag="x")
                    nc.sync.dma_start(xt[:],
                                      x_d[n, j * 128:(j + 1) * 128, :, :])
                    x_sb[n][j] = xt
                nc.sync.dma_start(qa1[n][:], z0_d[:])
                for j in range(NCH):
                    xt = x_sb[n][j]
                    m1 = mpool.tile([128, H, W], dt.bfloat16, name="m1", tag="m1")
                    m2 = mpool.tile([128, H, W], dt.bfloat16, name="m2", tag="m2")
                    m3 = mpool.tile([128, H, W], dt.bfloat16, name="m3", tag="m3")
                    ms = mpool.tile([128, H, W], dt.bfloat16, name="ms", tag="ms")
                    nc.vector.tensor_scalar(m1[:], xt[:], XQT[0], None,
                                            mybir.AluOpType.is_gt)
                    nc.vector.tensor_scalar(m2[:], xt[:], XQT[1], None,
                                            mybir.AluOpType.is_gt)
                    nc.vector.tensor_scalar(m3[:], xt[:], XQT[2], None,
                                            mybir.AluOpType.is_gt)
                    nc.vector.tensor_add(ms[:], m1[:], m2[:])
                    nc.vector.tensor_add(plane(qa1[n], j)[:, 1:1 + H, 1:1 + W],
                                         ms[:], m3[:])

            def dump_qa(qa):
                # debug stages: copy quantized planes out as f32
                for n in range(NP_):
                    for j in range(NCH):
                        o = opool.tile([128, H, W], dt.float32, name="od",
                                       tag="o")
                        nc.vector.tensor_copy(o[:], plane(qa[n], j)[:, 1:1 + H,
                                                                    1:1 + W])
                        nc.sync.dma_start(
                            y_d[n, j * 128:(j + 1) * 128, :, :], o[:])

            def conv_mms(ps, w_sb, qa_n, h, co):
                if fp8:
                    for k in range(9):
                        dy, dx = divmod(k, 3)
                        off = (h * HALF + dy) * PW + dx
                        nc.tensor.matmul(
                            ps[:, 0:RUN],
                            w_sb[:, 0:NCH, k, co * 128:(co + 1) * 128],
                            qa_n[:, 0:NCH, off:off + RUN],
                            start=(k == 0), stop=(k == 8),
                            perf_mode=mybir.MatmulPerfMode.DoubleRow,
                        )
                else:
                    for ci in range(NCH):
                        for k in range(9):
                            dy, dx = divmod(k, 3)
                            nc.tensor.matmul(
                                ps[:],
                                w_sb[:, ci, k, co * 128:(co + 1) * 128],
                                plane(qa_n, ci)[:, h * HALF + dy:
                                                h * HALF + dy + HALF,
                                                dx:dx + W],
                                start=(ci == 0 and k == 0),
                                stop=(ci == NCH - 1 and k == 8),
                            )

            def psum_tile(name):
                if fp8:
                    ps = pspool.tile([128, PSF], dt.float32, name=name,
                                     tag="ps")
                    psv = ps[:].rearrange("p (r c) -> p r c", c=PW)[:, :, 0:W]
                else:
                    ps = pspool.tile([128, HALF, W], dt.float32, name=name,
                                     tag="ps")
                    psv = ps[:]
                return ps, psv

            # conv1 -> bn1 -> relu -> quant; masks as ACT Signs vs folded
            # per-channel thresholds, summed into sign-sums on DVE
            def conv1_tile(n, h, co):
                        ps, psv = psum_tile("ps1")
                        conv_mms(ps, w1_sb, qa1[n], h, co)
                        m1 = mpool.tile([128, HALF, W], dt.bfloat16, name="e1",
                                        tag="e1")
                        m2 = mpool.tile([128, HALF, W], dt.bfloat16, name="e2",
                                        tag="e2")
                        m3 = mpool.tile([128, HALF, W], dt.bfloat16, name="e3",
                                        tag="e3")
                        ms = mpool.tile([128, HALF, W], dt.bfloat16, name="es",
                                        tag="es")
                        for k, mk in enumerate((m1, m2, m3)):
                            nc.scalar.activation(
                                mk[:], psv, mybir.ActivationFunctionType.Sign,
                                bias=t1_sb[:, co, k:k + 1])
                        nc.vector.tensor_add(ms[:], m1[:], m2[:])
                        nc.vector.tensor_add(
                            plane(qa2[n], co)[:, 1 + h * HALF:
                                              1 + h * HALF + HALF, 1:1 + W],
                            ms[:], m3[:])

            def conv1_image(n):
                nc.sync.dma_start(qa2[n][:], zm3_d[:])
                for h in range(2):
                    for co in range(NCH):
                        conv1_tile(n, h, co)

            # conv2 -> bn2 -> +residual -> relu -> out
            def conv2_image(n):
                for h in range(2):
                    for co in range(NCH):
                        ps, psv = psum_tile("ps2")
                        conv_mms(ps, w2_sb, qa2[n], h, co)
                        u = upool.tile([128, HALF, W], dt.float32, name="u",
                                       tag="u")
                        v = upool.tile([128, HALF, W], dt.float32, name="v",
                                       tag="v")
                        o = opool.tile([128, HALF, W], dt.float32, name="o",
                                       tag="o")
                        nc.scalar.activation(
                            u[:], psv,
                            mybir.ActivationFunctionType.Identity,
                            bias=b2_sb[:, co:co + 1], scale=s2_sb[:, co:co + 1])
                        nc.vector.tensor_add(
                            v[:], u[:],
                            x_sb[n][co][:, h * HALF:(h + 1) * HALF, :])
                        nc.scalar.activation(
                            o[:], v[:], mybir.ActivationFunctionType.Relu)
                        nc.sync.dma_start(
                            y_d[n, co * 128:(co + 1) * 128,
                                h * HALF:(h + 1) * HALF, :],
                            o[:])

            # software-pipelined emission: decouple engine queues by image
            for n in range(NP_):
                xq_image(n)
                if stage >= 2:
                    conv1_image(n)
                if stage >= 3 and n >= 1:
                    conv2_image(n - 1)
            if stage == 1:
                dump_qa(qa1)
            if stage == 2:
                dump_qa(qa2)
            if stage >= 3:
                conv2_image(NP_ - 1)

    nc.compile()
    return nc


_CACHED = None


def _get_program():
    global _CACHED
    if _CACHED is None:
        _CACHED = _build_program()
    return _CACHED


def kernel(x, w1, g1, b1, m1, v1, w2, g2, b2, m2, v2):
    _install_ntff_hook_shim()
    from concourse.bass_utils import run_bass_kernel_spmd

    x = np.asarray(x, np.float32)
    host = _host_arrays(w1, g1, b1, m1, v1, w2, g2, b2, m2, v2)

    xs = x.reshape(NCORES, NPER, C, H, W)
    in_maps = [{"x": np.ascontiguousarray(xs[c]), **host}
               for c in range(NCORES)]

    nc = _get_program()
    res = run_bass_kernel_spmd(
        nc, in_maps, core_ids=list(range(NCORES)),
        trace=bool(int(os.environ.get("KERNEL_TRACE", "0"))),
    )
    kernel.last_results = res
    y = np.concatenate([res.results[c]["y"][None] for c in range(NCORES)], 0)
    return np.ascontiguousarray(y.reshape(64, C, H, W).astype(np.float32))
